# revision 11
# baseline (speedup 1.0000x reference)
"""GQA attention kernel for 8 Trainium2 NeuronCores (Bass/Tile).

Sharding: data-parallel over batch (2) x tensor-parallel over head groups (4).
Core c: batch b=c//4, group g=c%4 (query heads 4g..4g+3, kv head g).
w_q/w_k/w_v column-parallel, w_o row-parallel; bf16 partial outputs are
ReduceScattered on-device over groups [[0..3],[4..7]]; host gather is a pure
concatenation + f32 cast.

Within-head feature order of w_q/w_k is permuted host-side to [re*32, im*32]
(instead of interleaved re/im) so RoPE uses contiguous 32-col slices; scores
are invariant since q and k share the permutation. rope_freqs are repacked
host-side to [T, 64] = [cos32 | sin32] and broadcast on-chip via stride-0 APs.

Design notes (v3):
- No PE transposes: all transposes are HWDGE xbar DMA-transposes (bf16), and
  ALL DMA traffic rides the sync ring so the ACT queue carries only the exps
  (a DMA's dependency wait would head-of-line-block the exps behind it).
- No warm-up collective: Tile chains DMA-lane semaphores round-robin, so a
  head-of-kernel collective rendezvous (~launch skew) stalls every later DMA.
  The j=0 ReduceScatter absorbs the skew while overlapped with compute.
- QK^T for the 2 heads of a pair run concurrently in PE row groups 0-63 /
  64-127 (K=64 each); kT is duplicated into both partition halves of plane 2
  of qkT. exp is one ACT instruction per (j, i, pair) over [128, 2, 512-o0].
- Softmax denominator via augmented-V ones column; per-head PSUM accumulator
  [128, 4, 65] packs 4 tq chunks in one bank (only chain c==0 sets start=True
  since a start clears has_written for the whole bank).

Hardcoded problem: B=2 T=2048 D=1024 n_heads=16 n_kv=4 d_head=64, causal,
RoPE theta=1e4 (freqs passed as input), scale=1/8.
"""

import numpy as np

import concourse.bass as bass
import concourse.tile as tile
from concourse import bacc, mybir
from concourse.bass_utils import run_bass_kernel_spmd

F32 = mybir.dt.float32
BF16 = mybir.dt.bfloat16
PT_DT = BF16            # dtype of exp'd probabilities (bf16 or float8e4)

B, T, D = 2, 2048, 1024
NH, NKV, DH = 16, 4, 64
HPC = NH // NKV          # query heads per core = 4
OC = HPC * DH            # per-core attn feature cols = 256
TB = T // 128            # 16 blocks of 128 rows
NJ = T // 512            # 4 tq-slices of 512
GROUPS = [[0, 1, 2, 3], [4, 5, 6, 7]]
SCALE = 1.0 / 8.0

_CACHE = {}


def _bcast(src, n, axis=1):
    """Insert a stride-0 broadcast dim of size n into a 2D AP at `axis`."""
    return bass.AP(tensor=src.tensor, offset=src.offset,
                   ap=src.ap[:axis] + [[0, n]] + src.ap[axis:])


def _emit(nc, tc, aps):
    x_ap, wq_ap, wk_ap, wv_ap, wo_ap, rope_ap, out_ap = aps
    import contextlib
    ctx = contextlib.ExitStack()
    with ctx:
        sing = ctx.enter_context(tc.tile_pool(name="sing", bufs=1))
        stage = ctx.enter_context(tc.tile_pool(name="stage", bufs=3))
        bstage = ctx.enter_context(tc.tile_pool(name="bstage", bufs=3))
        ropet = ctx.enter_context(tc.tile_pool(name="ropet", bufs=2))
        qrp = ctx.enter_context(tc.tile_pool(name="qrp", bufs=3))
        ptp = ctx.enter_context(tc.tile_pool(name="ptp", bufs=3))
        onatp = ctx.enter_context(tc.tile_pool(name="onatp", bufs=2))
        oTp = ctx.enter_context(tc.tile_pool(name="oTp", bufs=2))
        outsbp = ctx.enter_context(tc.tile_pool(name="outsbp", bufs=3))
        rcp = ctx.enter_context(tc.tile_pool(name="rcp", bufs=4))
        # PSUM: mm(2 x 1 bank) + st(2 x 2 banks) + oa(2 x 1 bank) = 8 banks
        psA = ctx.enter_context(tc.tile_pool(name="psA", bufs=2, space="PSUM"))
        psST = ctx.enter_context(tc.tile_pool(name="psST", bufs=2, space="PSUM"))
        psOA = ctx.enter_context(tc.tile_pool(name="psOA", bufs=2, space="PSUM"))
        dram = ctx.enter_context(tc.tile_pool(name="dram", bufs=1, space="DRAM"))

        # ---- persistent SBUF tensors
        wT = sing.tile([128, 8, 384], BF16)      # cols: 0:256 wq | 256:320 wk | 320:384 wv
        woT = sing.tile([128, 2, D], BF16)       # [o-chunk part, chunk, dout]
        xT = sing.tile([128, 8, T], BF16)        # [d-chunk part, chunk, t]
        qkT = sing.tile([128, 3, T], BF16)       # planes 0,1: q head pairs; 2: k (dup halves)
        vaug = sing.tile([128, TB, 65], BF16)    # col 64 = ones (rowsum trick)
        rope_c = sing.tile([128, TB, 64], F32)   # [cos32|sin32] compact
        xsall = sing.tile([128, TB, 1024], F32)  # full x prefetch
        nc.vector.memset(vaug[:], 1.0)
        nc.sync.dma_start(rope_c[:], rope_ap.rearrange("(tb p) f -> p tb f", p=128))
        _xr = x_ap.rearrange("(tb p) d -> p tb d", p=128)

        def load_x(tb):
            nc.sync.dma_start(xsall[:, tb, :], _xr[:, tb, :])

        # ---- qkv weights: load f32, cast bf16 (DVE), xbar-transpose (sync ring)
        for r in range(2):  # wq rows 256 -> 2 tiles
            wn = stage.tile([128, 1024], F32, tag="xst", name=f"wq{r}")
            nc.sync.dma_start(wn[:], wq_ap[128 * r:128 * (r + 1), :])
            wb = bstage.tile([128, 1024], BF16, tag="xbst", name=f"wqb{r}")
            nc.vector.tensor_copy(wb[:], wn[:])
            nc.sync.dma_start(wT[:, :, 128 * r:128 * (r + 1)], wb[:], transpose=True)
        wn = stage.tile([128, 1024], F32, tag="xst", name="wkv")
        nc.sync.dma_start(wn[0:64, :], wk_ap[:, :])
        nc.sync.dma_start(wn[64:128, :], wv_ap[:, :])
        wb = bstage.tile([128, 1024], BF16, tag="xbst", name="wkvb")
        nc.vector.tensor_copy(wb[:], wn[:])
        nc.sync.dma_start(wT[:, :, 256:384], wb[:], transpose=True)

        def load_wo():  # deferred: only needed at first out-projection
            for r in range(8):  # wo (1024, 256) -> 8 row tiles
                wn = stage.tile([128, 256], F32, tag="wost", name=f"wo{r}")
                nc.sync.dma_start(wn[:], wo_ap[128 * r:128 * (r + 1), :])
                wb = bstage.tile([128, 256], BF16, tag="wobst", name=f"wob{r}")
                nc.vector.tensor_copy(wb[:], wn[:])
                nc.sync.dma_start(woT[:, :, 128 * r:128 * (r + 1)], wb[:],
                                  transpose=True)

        partial = dram.tile([T, D], BF16)

        # ---- per t-block: load x, cast, transpose, QKV proj, rope, q/k transpose
        def phase12(tb):
            ts = slice(128 * tb, 128 * (tb + 1))
            xb = bstage.tile([128, 1024], BF16, tag="xbst", name="xb")
            nc.vector.tensor_copy(xb[:], xsall[:, tb, :])
            nc.sync.dma_start(xT[:, :, ts], xb[:], transpose=True)
            qkv = psA.tile([128, 512], F32, tag="mm", name="qkv")
            for dch in range(8):
                nc.tensor.matmul(qkv[:, 0:384], xT[:, dch, ts],
                                 wT[:, dch, :], start=(dch == 0), stop=(dch == 7))
            # rope over 5 groups (4 q heads + k), each [re*32 | im*32]
            qv = qkv[:, 0:320].rearrange("p (g h i) -> p g h i", g=5, h=2)
            re, im = qv[:, :, 0, :], qv[:, :, 1, :]
            cosv = _bcast(rope_c[:, tb, 0:32], 5)
            sinv = _bcast(rope_c[:, tb, 32:64], 5)
            t1 = ropet.tile([128, 5, 32], F32, tag="t1")
            t2 = ropet.tile([128, 5, 32], F32, tag="t2")
            t3 = ropet.tile([128, 5, 32], F32, tag="t3")
            t4 = ropet.tile([128, 5, 32], F32, tag="t4")
            nc.vector.tensor_mul(t1[:], re, cosv)
            nc.vector.tensor_mul(t2[:], im, sinv)
            nc.vector.tensor_mul(t3[:], re, sinv)
            nc.vector.tensor_mul(t4[:], im, cosv)
            qr = qrp.tile([128, 384], BF16, tag="qr")
            qro = qr[:, 0:320].rearrange("p (g h i) -> p g h i", g=5, h=2)
            nc.vector.tensor_sub(qro[:, :, 0, :], t1[:], t2[:])
            nc.vector.tensor_add(qro[:, :, 1, :], t3[:], t4[:])
            nc.vector.tensor_copy(qr[:, 320:384], qkv[:, 320:384])  # v (cast bf16)
            nc.vector.tensor_copy(vaug[:, tb, 0:64], qr[:, 320:384])
            # one transpose: planes [q01 | q23 | k+junk]
            nc.sync.dma_start(qkT[:, :, ts], qr[:], transpose=True)
            if tb % 4 == 3:
                js = slice(128 * (tb - 3), 128 * (tb + 1))
                nc.sync.dma_start(qkT[64:128, 2, js], qkT[0:64, 2, js])  # dup k

        # ---- attention for tq-slice j (tq 512j..512j+511), pair-major
        def phase3(j):
            nblk = 4 * j + 4
            onat = onatp.tile([128, 4, 256], BF16, tag="onat", name="onat")
            for p in range(2):
                oa = [psOA.tile([128, 4, 65], F32, tag="oa", name=f"oa{p}{hh}")
                      for hh in range(2)]
                for i in range(nblk):
                    o0 = max(0, 128 * i - 512 * j)
                    st = psST.tile([128, 2, 512], F32, tag="st", name="st")
                    for hh in range(2):
                        nc.tensor.matmul(
                            st[:, hh, o0:512],
                            qkT[64 * hh:64 * (hh + 1), 2, 128 * i:128 * (i + 1)],
                            qkT[64 * hh:64 * (hh + 1), p, 512 * j + o0:512 * (j + 1)],
                            start=True, stop=True)
                    pt = ptp.tile([128, 2, 512], PT_DT, tag="pt", name="pt")
                    nc.scalar.activation(pt[:, :, o0:512], st[:, :, o0:512],
                                         mybir.ActivationFunctionType.Exp, scale=SCALE)
                    if i >= 4 * j:  # diagonal block: zero tq < tk after exp
                        c = i - 4 * j
                        nc.gpsimd.affine_select(
                            out=pt[:, :, 128 * c:128 * (c + 1)],
                            in_=pt[:, :, 128 * c:128 * (c + 1)],
                            compare_op=mybir.AluOpType.is_ge,
                            fill=0.0, base=0,
                            pattern=[[0, 2], [1, 128]], channel_multiplier=-1)
                    for hh in range(2):
                        for c in range(4):
                            if i <= 4 * j + c:
                                nc.tensor.matmul(
                                    oa[hh][:, c, :],
                                    pt[:, hh, 128 * c:128 * (c + 1)],
                                    vaug[:, i, :],
                                    start=(i == 0 and c == 0),
                                    stop=(i == 4 * j + c),
                                    skip_group_check=True)
                # normalize pair p into onat chunks
                for hh in range(2):
                    h = 2 * p + hh
                    rc = rcp.tile([128, 4, 1], F32, tag="rc", name="rc")
                    nc.vector.reciprocal(rc[:], oa[hh][:, :, 64:65])
                    for c in range(4):
                        nc.vector.tensor_scalar_mul(
                            onat[:, c, DH * h:DH * (h + 1)],
                            oa[hh][:, c, 0:64], rc[:, c, :])
            # one O transpose for the whole slice: plane 2c+oc = oc-half of chunk c
            oTj = oTp.tile([128, 8, 128], BF16, tag="oTj", name="oTj")
            nc.sync.dma_start(oTj[:], onat[:].rearrange("p a b -> p (a b)"),
                              transpose=True)
            for c in range(4):
                tbw = 4 * j + c
                ws = slice(128 * tbw, 128 * (tbw + 1))
                for ns in range(2):
                    op = psA.tile([128, 512], F32, tag="mm", name=f"op{ns}")
                    for oc in range(2):
                        nc.tensor.matmul(op[:], oTj[:, 2 * c + oc, :],
                                         woT[:, oc, 512 * ns:512 * (ns + 1)],
                                         start=(oc == 0), stop=(oc == 1))
                    ob = outsbp.tile([128, 512], BF16, tag="ob", name="ob")
                    nc.vector.tensor_copy(ob[:], op[:])
                    nc.sync.dma_start(
                        partial[ws, 512 * ns:512 * (ns + 1)], ob[:])

        # software-pipelined x prefetch (3 deep) so the first QKV starts early
        load_x(0)
        load_x(1)
        load_x(2)
        for tb in range(TB):
            if tb + 3 < TB:
                load_x(tb + 3)
            phase12(tb)
            if tb == 3:
                load_wo()
        rsouts = []
        for j in range(NJ):
            phase3(j)
            # rows 512j..512j+512 complete -> ReduceScatter this quarter now
            rsout = dram.tile([128, D], BF16, name=f"rsout{j}")
            nc.gpsimd.collective_compute(
                "ReduceScatter", mybir.AluOpType.add, replica_groups=GROUPS,
                ins=[partial[512 * j:512 * (j + 1), :].opt()],
                outs=[rsout.opt()])
            rsouts.append(rsout)
        # final output DMAs forced to the very end of the sync ring: their
        # collective waits must not head-of-line-block compute-feeding DMAs
        # (the scheduler's cost model cannot see the collective's skew wait)
        for j in range(NJ):
            d = nc.sync.dma_start(out_ap[128 * j:128 * (j + 1), :], rsouts[j][:])
            d.ins.bass_priority = 1_000_000_000 + j


def _build():
    if "nc" in _CACHE:
        return _CACHE["nc"]
    nc = bacc.Bacc("TRN2", target_bir_lowering=False, debug=False, num_devices=8)
    x_ap = nc.dram_tensor("x", [T, D], F32, kind="ExternalInput").ap()
    wq_ap = nc.dram_tensor("wq", [OC, D], F32, kind="ExternalInput").ap()
    wk_ap = nc.dram_tensor("wk", [DH, D], F32, kind="ExternalInput").ap()
    wv_ap = nc.dram_tensor("wv", [DH, D], F32, kind="ExternalInput").ap()
    wo_ap = nc.dram_tensor("wo", [D, OC], F32, kind="ExternalInput").ap()
    rope_ap = nc.dram_tensor("rope", [T, DH], F32, kind="ExternalInput").ap()
    out_ap = nc.dram_tensor("out", [T // 4, D], BF16, kind="ExternalOutput").ap()
    with tile.TileContext(nc) as tc:
        _emit(nc, tc, (x_ap, wq_ap, wk_ap, wv_ap, wo_ap, rope_ap, out_ap))
    nc.compile()
    _CACHE["nc"] = nc
    return nc


# within-head feature permutation: [re components, im components]
_PERM = np.concatenate([np.arange(0, DH, 2), np.arange(1, DH, 2)])


def run(trace=False, **inputs):
    x = inputs["x"]
    rf = np.asarray(inputs["rope_freqs"], np.float32)       # (T, 32, 2)
    rope2 = np.ascontiguousarray(
        np.concatenate([rf[:, :, 0], rf[:, :, 1]], axis=1))  # (T, 64) = [cos|sin]
    w_q, w_k, w_v, w_o = (np.asarray(inputs[k], np.float32)
                          for k in ("w_q", "w_k", "w_v", "w_o"))
    nc = _build()
    in_maps = []
    for c in range(8):
        b, g = divmod(c, 4)
        wq_s = w_q[OC * g:OC * (g + 1)].reshape(HPC, DH, D)[:, _PERM, :]
        wk_s = w_k[DH * g:DH * (g + 1)][_PERM, :]
        in_maps.append({
            "x": np.ascontiguousarray(x[b], dtype=np.float32),
            "wq": np.ascontiguousarray(wq_s.reshape(OC, D)),
            "wk": np.ascontiguousarray(wk_s),
            "wv": np.ascontiguousarray(w_v[DH * g:DH * (g + 1)]),
            "wo": np.ascontiguousarray(w_o[:, OC * g:OC * (g + 1)]),
            "rope": rope2,
        })
    res = run_bass_kernel_spmd(nc, in_maps, core_ids=list(range(8)), trace=trace)
    out = np.empty((B, T, D), np.float32)
    for core in range(8):
        b, r = divmod(core, 4)
        for c in range(4):
            out[b, 512 * c + 128 * r:512 * c + 128 * (r + 1)] = \
                res.results[core]["out"][128 * c:128 * (c + 1)].astype(np.float32)
    return out, res


def kernel(**inputs):
    out, _ = run(trace=False, **inputs)
    return out


# revision 12
# speedup vs baseline: 1.1603x; 1.1603x over previous
"""GQA attention kernel for 8 Trainium2 NeuronCores (Bass/Tile).

Sharding: data-parallel over batch (2) x tensor-parallel over head groups (4).
Core c: batch b=c//4, group g=c%4 (query heads 4g..4g+3, kv head g).
w_q/w_k/w_v column-parallel, w_o row-parallel; bf16 partial outputs are
ReduceScattered on-device over groups [[0..3],[4..7]]; host gather is a pure
concatenation + f32 cast.

Within-head feature order of w_q/w_k is permuted host-side to [re*32, im*32]
(instead of interleaved re/im) so RoPE uses contiguous 32-col slices; scores
are invariant since q and k share the permutation. rope_freqs are repacked
host-side to [T, 64] = [cos32 | sin32] and broadcast on-chip via stride-0 APs.

Design notes (v3):
- No PE transposes: all transposes are HWDGE xbar DMA-transposes (bf16), and
  ALL DMA traffic rides the sync ring so the ACT queue carries only the exps
  (a DMA's dependency wait would head-of-line-block the exps behind it).
- No warm-up collective: Tile chains DMA-lane semaphores round-robin, so a
  head-of-kernel collective rendezvous (~launch skew) stalls every later DMA.
  The j=0 ReduceScatter absorbs the skew while overlapped with compute.
- QK^T for the 2 heads of a pair run concurrently in PE row groups 0-63 /
  64-127 (K=64 each); kT is duplicated into both partition halves of plane 2
  of qkT. exp is one ACT instruction per (j, i, pair) over [128, 2, 512-o0].
- Softmax denominator via augmented-V ones column; per-head PSUM accumulator
  [128, 4, 65] packs 4 tq chunks in one bank (only chain c==0 sets start=True
  since a start clears has_written for the whole bank).

Hardcoded problem: B=2 T=2048 D=1024 n_heads=16 n_kv=4 d_head=64, causal,
RoPE theta=1e4 (freqs passed as input), scale=1/8.
"""

import numpy as np

import concourse.bass as bass
import concourse.tile as tile
from concourse import bacc, mybir
from concourse.bass_utils import run_bass_kernel_spmd

F32 = mybir.dt.float32
BF16 = mybir.dt.bfloat16
PT_DT = BF16            # dtype of exp'd probabilities (bf16 or float8e4)

B, T, D = 2, 2048, 1024
NH, NKV, DH = 16, 4, 64
HPC = NH // NKV          # query heads per core = 4
OC = HPC * DH            # per-core attn feature cols = 256
TB = T // 128            # 16 blocks of 128 rows
NJ = T // 512            # 4 tq-slices of 512
GROUPS = [[0, 1, 2, 3], [4, 5, 6, 7]]
SCALE = 1.0 / 8.0

_CACHE = {}


def _bcast(src, n, axis=1):
    """Insert a stride-0 broadcast dim of size n into a 2D AP at `axis`."""
    return bass.AP(tensor=src.tensor, offset=src.offset,
                   ap=src.ap[:axis] + [[0, n]] + src.ap[axis:])


def _emit(nc, tc, aps):
    x_ap, wq_ap, wk_ap, wv_ap, wo_ap, rope_ap, out_ap = aps
    import contextlib
    ctx = contextlib.ExitStack()
    with ctx:
        sing = ctx.enter_context(tc.tile_pool(name="sing", bufs=1))
        stage = ctx.enter_context(tc.tile_pool(name="stage", bufs=3))
        bstage = ctx.enter_context(tc.tile_pool(name="bstage", bufs=3))
        ropet = ctx.enter_context(tc.tile_pool(name="ropet", bufs=2))
        qrp = ctx.enter_context(tc.tile_pool(name="qrp", bufs=3))
        ptp = ctx.enter_context(tc.tile_pool(name="ptp", bufs=3))
        onatp = ctx.enter_context(tc.tile_pool(name="onatp", bufs=2))
        oTp = ctx.enter_context(tc.tile_pool(name="oTp", bufs=2))
        outsbp = ctx.enter_context(tc.tile_pool(name="outsbp", bufs=3))
        rcp = ctx.enter_context(tc.tile_pool(name="rcp", bufs=4))
        # PSUM: mm(2 x 1 bank) + st(2 x 2 banks) + oa(2 x 1 bank) = 8 banks
        psA = ctx.enter_context(tc.tile_pool(name="psA", bufs=2, space="PSUM"))
        psST = ctx.enter_context(tc.tile_pool(name="psST", bufs=2, space="PSUM"))
        psOA = ctx.enter_context(tc.tile_pool(name="psOA", bufs=2, space="PSUM"))
        dram = ctx.enter_context(tc.tile_pool(name="dram", bufs=1, space="DRAM"))

        # ---- persistent SBUF tensors
        wT = sing.tile([128, 8, 384], BF16)      # cols: 0:256 wq | 256:320 wk | 320:384 wv
        woT = sing.tile([128, 2, D], BF16)       # [o-chunk part, chunk, dout]
        xT = sing.tile([128, 8, T], BF16)        # [d-chunk part, chunk, t]
        qkT = sing.tile([128, 3, T], BF16)       # planes 0,1: q head pairs; 2: k (dup halves)
        vaug = sing.tile([128, TB, 65], BF16)    # col 64 = ones (rowsum trick)
        rope_c = sing.tile([128, TB, 128], F32)  # [cos|sin|sin|cos]
        xsall = sing.tile([128, TB, 1024], F32)  # full x prefetch
        nc.vector.memset(vaug[:], 1.0)
        nc.sync.dma_start(rope_c[:], rope_ap.rearrange("(tb p) f -> p tb f", p=128))
        _xr = x_ap.rearrange("(tb p) d -> p tb d", p=128)

        def load_x(q):  # quarter: 4 t-blocks = 2MB
            nc.sync.dma_start(xsall[:, 4 * q:4 * (q + 1), :],
                              _xr[:, 4 * q:4 * (q + 1), :])

        # ---- qkv weights: load f32, cast bf16 (DVE), xbar-transpose (sync ring)
        for r in range(2):  # wq rows 256 -> 2 tiles
            wn = stage.tile([128, 1024], F32, tag="xst", name=f"wq{r}")
            nc.sync.dma_start(wn[:], wq_ap[128 * r:128 * (r + 1), :])
            wb = bstage.tile([128, 1024], BF16, tag="xbst", name=f"wqb{r}")
            nc.vector.tensor_copy(wb[:], wn[:])
            nc.sync.dma_start(wT[:, :, 128 * r:128 * (r + 1)], wb[:], transpose=True)
        wn = stage.tile([128, 1024], F32, tag="xst", name="wkv")
        nc.sync.dma_start(wn[0:64, :], wk_ap[:, :])
        nc.sync.dma_start(wn[64:128, :], wv_ap[:, :])
        wb = bstage.tile([128, 1024], BF16, tag="xbst", name="wkvb")
        nc.vector.tensor_copy(wb[:], wn[:])
        nc.sync.dma_start(wT[:, :, 256:384], wb[:], transpose=True)

        def load_wo():  # deferred: only needed at first out-projection
            for r in range(8):  # wo (1024, 256) -> 8 row tiles
                wn = stage.tile([128, 256], F32, tag="wost", name=f"wo{r}")
                nc.sync.dma_start(wn[:], wo_ap[128 * r:128 * (r + 1), :])
                wb = bstage.tile([128, 256], BF16, tag="wobst", name=f"wob{r}")
                nc.vector.tensor_copy(wb[:], wn[:])
                nc.sync.dma_start(woT[:, :, 128 * r:128 * (r + 1)], wb[:],
                                  transpose=True)

        partial = dram.tile([T, D], BF16)

        # ---- per t-block: load x, cast, transpose, QKV proj, rope, q/k transpose
        def phase12(tb):
            ts = slice(128 * tb, 128 * (tb + 1))
            xb = bstage.tile([128, 1024], BF16, tag="xbst", name="xb")
            nc.scalar.copy(xb[:], xsall[:, tb, :])  # cast on ACT (idle in proj)
            nc.sync.dma_start(xT[:, :, ts], xb[:], transpose=True)
            qkv = psA.tile([128, 512], F32, tag="mm", name="qkv")
            for dch in range(8):
                nc.tensor.matmul(qkv[:, 0:384], xT[:, dch, ts],
                                 wT[:, dch, :], start=(dch == 0), stop=(dch == 7))
            # rope over 5 groups (4 q heads + k), each [re*32 | im*32]:
            # t1 = [re*cos | im*sin], t2 = [re*sin | im*cos]
            qv = qkv[:, 0:320].rearrange("p (g f) -> p g f", g=5)
            cs = _bcast(rope_c[:, tb, 0:64], 5)
            sc = _bcast(rope_c[:, tb, 64:128], 5)
            t1 = ropet.tile([128, 5, 64], F32, tag="t1")
            t2 = ropet.tile([128, 5, 64], F32, tag="t2")
            nc.vector.tensor_mul(t1[:], qv, cs)
            nc.vector.tensor_mul(t2[:], qv, sc)
            qr = qrp.tile([128, 384], BF16, tag="qr")
            qro = qr[:, 0:320].rearrange("p (g h i) -> p g h i", g=5, h=2)
            nc.vector.tensor_sub(qro[:, :, 0, :], t1[:, :, 0:32], t1[:, :, 32:64])
            nc.vector.tensor_add(qro[:, :, 1, :], t2[:, :, 0:32], t2[:, :, 32:64])
            nc.vector.tensor_copy(qr[:, 320:384], qkv[:, 320:384])  # v (cast bf16)
            nc.vector.tensor_copy(vaug[:, tb, 0:64], qr[:, 320:384])
            # one transpose: planes [q01 | q23 | k+junk]
            nc.sync.dma_start(qkT[:, :, ts], qr[:], transpose=True)
            if tb % 4 == 3:
                js = slice(128 * (tb - 3), 128 * (tb + 1))
                nc.sync.dma_start(qkT[64:128, 2, js], qkT[0:64, 2, js])  # dup k

        # ---- attention for tq-slice j (tq 512j..512j+511), pair-major
        def phase3(j):
            nblk = 4 * j + 4
            onat = onatp.tile([128, 4, 256], BF16, tag="onat", name="onat")
            for p in range(2):
                oa = [psOA.tile([128, 4, 65], F32, tag="oa", name=f"oa{p}{hh}")
                      for hh in range(2)]
                for i in range(nblk):
                    o0 = max(0, 128 * i - 512 * j)
                    st = psST.tile([128, 2, 512], F32, tag="st", name="st")
                    for hh in range(2):
                        nc.tensor.matmul(
                            st[:, hh, o0:512],
                            qkT[64 * hh:64 * (hh + 1), 2, 128 * i:128 * (i + 1)],
                            qkT[64 * hh:64 * (hh + 1), p, 512 * j + o0:512 * (j + 1)],
                            start=True, stop=True)
                    pt = ptp.tile([128, 2, 512], PT_DT, tag="pt", name="pt")
                    nc.scalar.activation(pt[:, :, o0:512], st[:, :, o0:512],
                                         mybir.ActivationFunctionType.Exp, scale=SCALE)
                    if i >= 4 * j:  # diagonal block: zero tq < tk after exp
                        c = i - 4 * j
                        nc.gpsimd.affine_select(
                            out=pt[:, :, 128 * c:128 * (c + 1)],
                            in_=pt[:, :, 128 * c:128 * (c + 1)],
                            compare_op=mybir.AluOpType.is_ge,
                            fill=0.0, base=0,
                            pattern=[[0, 2], [1, 128]], channel_multiplier=-1)
                    for hh in range(2):
                        for c in range(4):
                            if i <= 4 * j + c:
                                nc.tensor.matmul(
                                    oa[hh][:, c, :],
                                    pt[:, hh, 128 * c:128 * (c + 1)],
                                    vaug[:, i, :],
                                    start=(i == 0 and c == 0),
                                    stop=(i == 4 * j + c),
                                    skip_group_check=True)
                # normalize pair p into onat chunks
                for hh in range(2):
                    h = 2 * p + hh
                    rc = rcp.tile([128, 4, 1], F32, tag="rc", name="rc")
                    nc.vector.reciprocal(rc[:], oa[hh][:, :, 64:65])
                    for c in range(4):
                        nc.vector.tensor_scalar_mul(
                            onat[:, c, DH * h:DH * (h + 1)],
                            oa[hh][:, c, 0:64], rc[:, c, :])
            # one O transpose for the whole slice: plane 2c+oc = oc-half of chunk c
            oTj = oTp.tile([128, 8, 128], BF16, tag="oTj", name="oTj")
            nc.sync.dma_start(oTj[:], onat[:].rearrange("p a b -> p (a b)"),
                              transpose=True)
            for c in range(4):
                tbw = 4 * j + c
                ws = slice(128 * tbw, 128 * (tbw + 1))
                for ns in range(2):
                    op = psA.tile([128, 512], F32, tag="mm", name=f"op{ns}")
                    for oc in range(2):
                        nc.tensor.matmul(op[:], oTj[:, 2 * c + oc, :],
                                         woT[:, oc, 512 * ns:512 * (ns + 1)],
                                         start=(oc == 0), stop=(oc == 1))
                    ob = outsbp.tile([128, 512], BF16, tag="ob", name="ob")
                    nc.vector.tensor_copy(ob[:], op[:])
                    nc.sync.dma_start(
                        partial[ws, 512 * ns:512 * (ns + 1)], ob[:])

        # x prefetch in quarters, pipelined ahead of the per-block work
        load_x(0)
        load_x(1)
        for tb in range(TB):
            if tb % 4 == 0 and tb // 4 + 2 < 4:
                load_x(tb // 4 + 2)
            phase12(tb)
            if tb == 3:
                load_wo()
        rsouts = []
        for j in range(NJ):
            phase3(j)
            # rows 512j..512j+512 complete -> ReduceScatter this quarter now
            rsout = dram.tile([128, D], BF16, name=f"rsout{j}")
            nc.gpsimd.collective_compute(
                "ReduceScatter", mybir.AluOpType.add, replica_groups=GROUPS,
                ins=[partial[512 * j:512 * (j + 1), :].opt()],
                outs=[rsout.opt()])
            rsouts.append(rsout)
        # final output DMAs forced to the very end of the sync ring: their
        # collective waits must not head-of-line-block compute-feeding DMAs
        # (the scheduler's cost model cannot see the collective's skew wait)
        for j in range(NJ):
            d = nc.sync.dma_start(out_ap[128 * j:128 * (j + 1), :], rsouts[j][:])
            d.ins.bass_priority = 1_000_000_000 + j


def _build():
    if "nc" in _CACHE:
        return _CACHE["nc"]
    nc = bacc.Bacc("TRN2", target_bir_lowering=False, debug=False, num_devices=8)
    x_ap = nc.dram_tensor("x", [T, D], F32, kind="ExternalInput").ap()
    wq_ap = nc.dram_tensor("wq", [OC, D], F32, kind="ExternalInput").ap()
    wk_ap = nc.dram_tensor("wk", [DH, D], F32, kind="ExternalInput").ap()
    wv_ap = nc.dram_tensor("wv", [DH, D], F32, kind="ExternalInput").ap()
    wo_ap = nc.dram_tensor("wo", [D, OC], F32, kind="ExternalInput").ap()
    rope_ap = nc.dram_tensor("rope", [T, 2 * DH], F32, kind="ExternalInput").ap()
    out_ap = nc.dram_tensor("out", [T // 4, D], BF16, kind="ExternalOutput").ap()
    with tile.TileContext(nc) as tc:
        _emit(nc, tc, (x_ap, wq_ap, wk_ap, wv_ap, wo_ap, rope_ap, out_ap))
    nc.compile()
    _CACHE["nc"] = nc
    return nc


# within-head feature permutation: [re components, im components]
_PERM = np.concatenate([np.arange(0, DH, 2), np.arange(1, DH, 2)])


def run(trace=False, **inputs):
    x = inputs["x"]
    rf = np.asarray(inputs["rope_freqs"], np.float32)       # (T, 32, 2)
    cos, sin = rf[:, :, 0], rf[:, :, 1]
    rope2 = np.ascontiguousarray(
        np.concatenate([cos, sin, sin, cos], axis=1))        # (T, 128)
    w_q, w_k, w_v, w_o = (np.asarray(inputs[k], np.float32)
                          for k in ("w_q", "w_k", "w_v", "w_o"))
    nc = _build()
    in_maps = []
    for c in range(8):
        b, g = divmod(c, 4)
        wq_s = w_q[OC * g:OC * (g + 1)].reshape(HPC, DH, D)[:, _PERM, :]
        wk_s = w_k[DH * g:DH * (g + 1)][_PERM, :]
        in_maps.append({
            "x": np.ascontiguousarray(x[b], dtype=np.float32),
            "wq": np.ascontiguousarray(wq_s.reshape(OC, D)),
            "wk": np.ascontiguousarray(wk_s),
            "wv": np.ascontiguousarray(w_v[DH * g:DH * (g + 1)]),
            "wo": np.ascontiguousarray(w_o[:, OC * g:OC * (g + 1)]),
            "rope": rope2,
        })
    res = run_bass_kernel_spmd(nc, in_maps, core_ids=list(range(8)), trace=trace)
    out = np.empty((B, T, D), np.float32)
    for core in range(8):
        b, r = divmod(core, 4)
        for c in range(4):
            out[b, 512 * c + 128 * r:512 * c + 128 * (r + 1)] = \
                res.results[core]["out"][128 * c:128 * (c + 1)].astype(np.float32)
    return out, res


def kernel(**inputs):
    out, _ = run(trace=False, **inputs)
    return out
